# revision 10
# baseline (speedup 1.0000x reference)
"""Trainium2 Bass kernel for nn_ContrastiveLoss_82300163326281.

Strategy (8 NeuronCores, SPMD, no collectives):
  - Host rotates the embedding rows per core (core k gets roll(emb, -1024k))
    so every core runs the *same* program on its local rows 0..1023 while the
    full matrix column space is identical up to a permutation (row reductions
    are permutation invariant).
  - Device, per core:
      phase 0: normalize all B rows (ACT square-accum -> sqrt -> 1/x -> scale)
               and PE-transpose into a resident zT panel [2x128, B] (f32).
      main:    for each 128-row block x 2048-col chunk:
                 fp32r matmul -> PSUM (raw dots v)
                 ACT: E = exp(v*invtemp - c) with accum -> rowsum(E)
                 DVE: tensor_tensor_reduce -> rowsum(v*E)
                 DVE: rowwise min/max of v, skipping the 256-wide diagonal
                      window [128rb, 128rb+256) that contains the diagonal
                      and all K=8 positives  (c = 1/temp = the row max, since
                      the diagonal of a cosine-similarity matrix dominates)
                 DMA: ship the raw v window [128,256] to DRAM
  - Host finish (exact, f64): per-row masked min/max merge (device outside-
    window extremes + host window scan), global neg_min/neg_max, affine
    decomposition of the 'inverse_sim' weights  w = a*s' + b_r  so that
      sum_j w_j e^{s'_j} = a*sum(s'E) + b_r*sum(E) (+ pos/diag corrections),
    positive log-probs from the shipped windows, weighted mean.

Self-contained: hardcodes shapes; falls back to a pure-numpy replica of the
reference if the positive-index structure is not the expected banded pattern.
"""

import os
import sys

import numpy as np

sys.path.insert(0, "/opt/trn_rl_repo")

B = 8192
D = 256
K = 8
NCORES = 8
ROWS = B // NCORES          # 1024 rows per core
RB = ROWS // 128            # 8 row blocks per core
CHUNK = 2048
NCH = B // CHUNK            # 4 column chunks
WIN = 256                   # diagonal window width (>= 128 + K + 1)
EPS = 1e-8

_state = {}


# --------------------------------------------------------------------------
# device program
# --------------------------------------------------------------------------

def _build_program(invtemp: float, negc: float, use_ttr=True, use_f32r=True, do_main=True):
    from contextlib import ExitStack

    import concourse.bass as bass  # noqa: F401
    import concourse.mybir as mybir
    from concourse import bacc, tile

    f32 = mybir.dt.float32
    f32r = mybir.dt.float32r
    AF = mybir.ActivationFunctionType
    ALU = mybir.AluOpType
    AX = mybir.AxisListType

    nc = bacc.Bacc(
        "TRN2",
        target_bir_lowering=False,
        debug=False,
        num_devices=NCORES,
    )
    emb = nc.dram_tensor("emb", [B, D], f32, kind="ExternalInput").ap()
    stats = nc.dram_tensor("stats", [128, RB * 8], f32, kind="ExternalOutput").ap()
    wins = nc.dram_tensor("wins", [128, RB * WIN], f32, kind="ExternalOutput").ap()

    with tile.TileContext(nc) as tc, ExitStack() as ctx:
        const = ctx.enter_context(tc.tile_pool(name="const", bufs=1))
        ones = const.tile([128, 128], f32, tag="ones", name="ones")
        ident = const.tile([128, 128], f32, tag="ident", name="ident")
        ebias = const.tile([128, 1], f32, tag="ebias", name="ebias")
        nc.gpsimd.memset(ones[:], 1.0)
        nc.gpsimd.affine_select(
            ident[:],
            ones[:],
            pattern=[[1, 128]],
            compare_op=ALU.is_equal,
            fill=0.0,
            base=0,
            channel_multiplier=-1,
        )
        nc.gpsimd.memset(ebias[:], negc)

        ztp = ctx.enter_context(tc.tile_pool(name="ztp", bufs=1))
        # zt[:, 0:B] = dims 0..127 (chunk 0), zt[:, B:2B] = dims 128..255
        zt = ztp.tile([128, 2 * B], f32r if use_f32r else f32, tag="zt", name="zt")

        ep = ctx.enter_context(tc.tile_pool(name="ep", bufs=2))
        zp = ctx.enter_context(tc.tile_pool(name="zp", bufs=3))
        small = ctx.enter_context(tc.tile_pool(name="small", bufs=4))
        psum = ctx.enter_context(tc.tile_pool(name="psum", bufs=2, space="PSUM"))
        Epool = ctx.enter_context(tc.tile_pool(name="Epool", bufs=3))
        upool = ctx.enter_context(tc.tile_pool(name="upool", bufs=2))
        accp = ctx.enter_context(tc.tile_pool(name="accp", bufs=RB))
        outp = ctx.enter_context(tc.tile_pool(name="outp", bufs=1))

        stats_sb = outp.tile([128, RB * 8], f32, tag="stats_sb", name="stats_sb")
        nc.gpsimd.memset(stats_sb[:], 0.0)

        emb_r = emb.rearrange("(a p) d -> p a d", p=128)  # [128, 64, 256]

        def phase0(c):
            # normalize + transpose z row-tiles [16c, 16c+16)
            eg = ep.tile([128, 16 * D], f32, tag="eg", name=f"eg{c}")
            nc.sync.dma_start(
                out=eg[:].rearrange("p (a d) -> p a d", d=D),
                in_=emb_r[:, 16 * c : 16 * c + 16, :],
            )
            for j in range(8):  # 2 z-tiles per psum tile (4 banks)
                pt = psum.tile([128, CHUNK], f32, tag="pt", name=f"tp{c}_{j}")
                for h in range(2):
                    t = 16 * c + 2 * j + h
                    et = eg[:, (2 * j + h) * D : (2 * j + h + 1) * D]
                    sq = zp.tile([128, D], f32, tag="sq", name=f"sq{t}")
                    n2 = small.tile([128, 1], f32, tag="n2", name=f"n2_{t}")
                    nc.scalar.activation(sq[:], et, AF.Square, accum_out=n2[:])
                    nrm = small.tile([128, 1], f32, tag="nrm", name=f"nrm{t}")
                    nc.scalar.activation(nrm[:], n2[:], AF.Sqrt)
                    rn = small.tile([128, 1], f32, tag="rn", name=f"rn{t}")
                    nc.vector.reciprocal(rn[:], nrm[:])
                    zr = zp.tile([128, D], f32, tag="zr", name=f"zr{t}")
                    nc.scalar.activation(zr[:], et, AF.Copy, scale=rn[:])
                    # one transpose per PSUM bank (bank = one zero region)
                    nc.tensor.matmul(
                        pt[:, 1024 * h : 1024 * h + 128],
                        lhsT=zr[:, 0:128],
                        rhs=ident[:],
                        is_transpose=True,
                        start=True,
                        stop=True,
                    )
                    nc.tensor.matmul(
                        pt[:, 1024 * h + 512 : 1024 * h + 640],
                        lhsT=zr[:, 128:256],
                        rhs=ident[:],
                        is_transpose=True,
                        start=True,
                        stop=True,
                    )
                # evacuate PSUM -> zt panel (DMA cannot read PSUM); alternate
                # DVE / ACT so neither engine eats the whole copy cost
                t0 = 16 * c + 2 * j
                for h in range(2):
                    t = t0 + h
                    src0 = pt[:, 1024 * h : 1024 * h + 128]
                    src1 = pt[:, 1024 * h + 512 : 1024 * h + 640]
                    d0 = zt[:, 128 * t : 128 * t + 128]
                    d1 = zt[:, B + 128 * t : B + 128 * t + 128]
                    if t % 2 == 0:
                        nc.vector.tensor_copy(d0, src0)
                        nc.scalar.copy(d1, src1)
                    else:
                        nc.scalar.copy(d0, src0)
                        nc.vector.tensor_copy(d1, src1)

        def main_block(rb, c):
            pt = psum.tile([128, CHUNK], f32, tag="pt", name=f"pt{rb}_{c}")
            l0 = zt[:, 128 * rb : 128 * rb + 128]
            l1 = zt[:, B + 128 * rb : B + 128 * rb + 128]
            for b in range(CHUNK // 512):
                col = CHUNK * c + 512 * b
                nc.tensor.matmul(
                    pt[:, 512 * b : 512 * b + 512],
                    lhsT=l0,
                    rhs=zt[:, col : col + 512],
                    start=True,
                    stop=False,
                )
                nc.tensor.matmul(
                    pt[:, 512 * b : 512 * b + 512],
                    lhsT=l1,
                    rhs=zt[:, B + col : B + col + 512],
                    start=False,
                    stop=True,
                )

            se, su, mn, mx = _state["acc"][rb]
            E = Epool.tile([128, CHUNK], f32, tag="E", name=f"E{rb}_{c}")
            nc.scalar.activation(
                E[:],
                pt[:],
                AF.Exp,
                bias=ebias[:],
                scale=float(invtemp),
                accum_out=se[:, c : c + 1],
            )
            u = upool.tile([128, CHUNK], f32, tag="u", name=f"u{rb}_{c}")
            if use_ttr:
                nc.vector.scalar_tensor_tensor(
                    out=u[:],
                    in0=pt[:],
                    scalar=1.0,
                    in1=E[:],
                    op0=ALU.bypass,
                    op1=ALU.mult,
                    accum_out=su[:, c : c + 1],
                )
            else:
                nc.vector.tensor_tensor(u[:], pt[:], E[:], op=ALU.mult)
                nc.vector.reduce_sum(su[:, c : c + 1], u[:], axis=AX.X)

            # min/max of raw v, excluding the diagonal window on chunk 0
            if c == 0:
                o = 128 * rb
                pieces = []
                if rb > 0:
                    pieces.append((0, o))
                pieces.append((o + WIN, CHUNK - (o + WIN)))
                wstage = Epool.tile(
                    [128, WIN], f32, tag="wstage", name=f"wstage{rb}", bufs=2
                )
                nc.scalar.copy(wstage[:], pt[:, o : o + WIN])
                nc.sync.dma_start(
                    out=wins[:, WIN * rb : WIN * rb + WIN],
                    in_=wstage[:],
                )
            else:
                pieces = [(0, CHUNK)]
            pidx = _state["pidx"][rb]
            for (a, w) in pieces:
                nc.vector.tensor_reduce(
                    mn[:, pidx : pidx + 1], pt[:, a : a + w], axis=AX.X, op=ALU.min
                )
                nc.vector.tensor_reduce(
                    mx[:, pidx : pidx + 1], pt[:, a : a + w], axis=AX.X, op=ALU.max
                )
                pidx += 1
            _state["pidx"][rb] = pidx

        def finish_block(rb):
            se, su, mn, mx = _state["acc"][rb]
            npieces = _state["pidx"][rb]
            nc.vector.tensor_reduce(
                stats_sb[:, 8 * rb + 0 : 8 * rb + 1], se[:], axis=AX.X, op=ALU.add
            )
            nc.vector.tensor_reduce(
                stats_sb[:, 8 * rb + 1 : 8 * rb + 1 + 1], su[:], axis=AX.X, op=ALU.add
            )
            nc.vector.tensor_reduce(
                stats_sb[:, 8 * rb + 2 : 8 * rb + 3],
                mn[:, 0:npieces],
                axis=AX.X,
                op=ALU.min,
            )
            nc.vector.tensor_reduce(
                stats_sb[:, 8 * rb + 3 : 8 * rb + 4],
                mx[:, 0:npieces],
                axis=AX.X,
                op=ALU.max,
            )

        # per-rowblock accumulators
        _state["acc"] = {}
        _state["pidx"] = {}
        for rb in range(RB):
            se = accp.tile([128, NCH], f32, tag="se", name=f"se{rb}")
            su = accp.tile([128, NCH], f32, tag="su", name=f"su{rb}")
            mn = accp.tile([128, 5], f32, tag="mn", name=f"mn{rb}")
            mx = accp.tile([128, 5], f32, tag="mx", name=f"mx{rb}")
            _state["acc"][rb] = (se, su, mn, mx)
            _state["pidx"][rb] = 0

        phase0(0)
        for c in range(NCH):
            if c + 1 < NCH:
                phase0(c + 1)
            if do_main:
                for rb in range(RB):
                    main_block(rb, c)
        if do_main:
            for rb in range(RB):
                finish_block(rb)

        nc.sync.dma_start(out=stats, in_=stats_sb[:])

        _state.pop("acc", None)
        _state.pop("pidx", None)

    nc.compile()
    return nc


# --------------------------------------------------------------------------
# runners
# --------------------------------------------------------------------------

def _get_program(invtemp: float, negc: float):
    key = ("prog", float(invtemp), float(negc))
    if key not in _state:
        _state[key] = _build_program(invtemp, negc)
    return _state[key]


def _run_device_stock(nc, in_maps):
    from concourse.bass_utils import run_bass_kernel_spmd

    res = run_bass_kernel_spmd(nc, in_maps, list(range(NCORES)))
    _state["last_results"] = res
    return res.results


def _make_cached_runner(nc):
    """Vendored multi-core tail of bass2jax.run_bass_via_pjrt, but keeping the
    jitted callable so repeated invocations (for timing) do not recompile."""
    import jax
    import concourse.mybir as mybir
    from jax.sharding import Mesh, PartitionSpec
    from concourse.bass2jax import (
        _bass_exec_p,
        install_neuronx_cc_hook,
        partition_id_tensor,
    )

    try:
        from jax.experimental.shard_map import shard_map
    except Exception:  # newer jax
        from jax import shard_map  # type: ignore

    install_neuronx_cc_hook()

    partition_name = nc.partition_id_tensor.name if nc.partition_id_tensor else None
    in_names, out_names, out_avals, zero_outs = [], [], [], []
    for alloc in nc.m.functions[0].allocations:
        if not isinstance(alloc, mybir.MemoryLocationSet):
            continue
        name = alloc.memorylocations[0].name
        if alloc.kind == "ExternalInput":
            if name != partition_name:
                in_names.append(name)
        elif alloc.kind == "ExternalOutput":
            out_names.append(name)
            shape = tuple(alloc.tensor_shape)
            dtype = mybir.dt.np(alloc.dtype)
            out_avals.append(jax.core.ShapedArray(shape, dtype))
            zero_outs.append(np.zeros(shape, dtype))
    n_params = len(in_names)
    all_names = in_names + out_names
    if partition_name is not None:
        all_names = all_names + [partition_name]
    donate = tuple(range(n_params, n_params + len(out_names)))

    def _body(*args):
        operands = list(args)
        if partition_name is not None:
            operands.append(partition_id_tensor())
        outs = _bass_exec_p.bind(
            *operands,
            out_avals=tuple(out_avals),
            in_names=tuple(all_names),
            out_names=tuple(out_names),
            lowering_input_output_aliases=(),
            sim_require_finite=True,
            sim_require_nnan=True,
            nc=nc,
        )
        return tuple(outs)

    devices = jax.devices()[:NCORES]
    mesh = Mesh(np.asarray(devices), ("core",))
    n_out = len(out_names)
    sharded = jax.jit(
        shard_map(
            _body,
            mesh=mesh,
            in_specs=(PartitionSpec("core"),) * (n_params + n_out),
            out_specs=(PartitionSpec("core"),) * n_out,
            check_rep=False,
        ),
        donate_argnums=donate,
        keep_unused=True,
    )

    def run(in_maps):
        concat_in = [
            np.concatenate([np.asarray(m[nm]) for m in in_maps], axis=0)
            for nm in in_names
        ]
        concat_zeros = [
            np.zeros((NCORES * z.shape[0], *z.shape[1:]), z.dtype) for z in zero_outs
        ]
        out_arrs = sharded(*concat_in, *concat_zeros)
        return [
            {
                nm: np.asarray(out_arrs[i]).reshape(NCORES, *out_avals[i].shape)[c]
                for i, nm in enumerate(out_names)
            }
            for c in range(NCORES)
        ]

    return run


def _run_device(nc, in_maps):
    if os.environ.get("KERNEL_FAST_RUNNER"):
        key = ("runner", id(nc))
        if key not in _state:
            _state[key] = _make_cached_runner(nc)
        return _state[key](in_maps)
    return _run_device_stock(nc, in_maps)


# --------------------------------------------------------------------------
# host finish
# --------------------------------------------------------------------------

def _numpy_reference(emb, pos_vals, temperature, pos_row, pos_col):
    """Exact fallback replica of the reference (used only if the positive
    index pattern is not the expected banded structure)."""
    n = emb.shape[0]
    norm = np.sqrt((emb.astype(np.float32) ** 2).sum(1, keepdims=True))
    z = emb / np.maximum(norm, np.float32(1e-12))
    temp = np.float32(np.log1p(np.exp(np.float64(temperature))))
    sim = (z @ z.T) / temp
    sim = sim - sim.max(axis=1, keepdims=True)
    posd = np.zeros((n, n), bool)
    posd[pos_row, pos_col] = True
    negm = ~posd & ~np.eye(n, dtype=bool)
    pos_w = 1.0 - pos_vals
    pos_w = (pos_w - pos_w.min()) / (pos_w.max() - pos_w.min() + np.float32(EPS))
    neg_min = sim[negm].min()
    neg_max = sim[negm].max()
    neg_w = (sim - neg_min) / (neg_max - neg_min + np.float32(EPS)) + 1.0
    logw = np.where(negm, np.log(neg_w), 0.0).astype(np.float32)
    a = (sim + logw).astype(np.float64)
    lse = np.log(np.exp(a).sum(1))
    pl = sim[pos_row, pos_col].astype(np.float64) - lse[pos_row]
    return np.float32(-np.mean(pl * pos_w.astype(np.float64)))


def kernel(**inputs):
    emb = np.ascontiguousarray(np.asarray(inputs["embeddings"], dtype=np.float32))
    pos_vals = np.asarray(inputs["pos_vals"], dtype=np.float32)
    temperature = np.asarray(inputs["temperature"], dtype=np.float32)
    pos_row = np.asarray(inputs["pos_row"]).astype(np.int64)
    pos_col = np.asarray(inputs["pos_col"]).astype(np.int64)

    rr = np.repeat(np.arange(B, dtype=np.int64), K)
    oo = np.tile(np.arange(1, K + 1, dtype=np.int64), B)
    structured = (
        emb.shape == (B, D)
        and pos_row.shape == (B * K,)
        and np.array_equal(pos_row, rr)
        and np.array_equal(pos_col, (rr + oo) % B)
    )
    if not structured:
        return _numpy_reference(emb, pos_vals, temperature, pos_row, pos_col)

    temp = float(np.log1p(np.exp(np.float64(temperature))))
    invtemp = 1.0 / np.float32(temp)  # f32 to match device immediates
    invtemp = float(np.float32(invtemp))
    c = invtemp  # row max == diagonal == 1/temp
    negc = float(np.float32(-c))

    nc = _get_program(invtemp, negc)
    in_maps = [
        {"emb": np.roll(emb, -ROWS * k, axis=0)} for k in range(NCORES)
    ]
    results = _run_device(nc, in_maps)

    # ---- host finish (f64) ----
    it = np.float64(invtemp)
    cc = np.float64(c)

    sumE = np.empty(B)
    sumU = np.empty(B)
    row_min = np.empty(B)
    row_max = np.empty(B)
    m = np.empty(B)
    Wv = np.empty((B, WIN))

    ridx = np.arange(128)
    for k in range(NCORES):
        stats = results[k]["stats"].astype(np.float64)  # [128, RB*8]
        wins = results[k]["wins"].astype(np.float64)    # [128, RB*WIN]
        for rb in range(RB):
            g0 = ROWS * k + 128 * rb
            s = stats[:, 8 * rb : 8 * rb + 8]
            W = wins[:, WIN * rb : WIN * rb + WIN]  # [128, 256] raw v
            sumE[g0 : g0 + 128] = s[:, 0]
            sumU[g0 : g0 + 128] = s[:, 1]
            # masked min/max inside window: exclude relative cols r..r+K
            Wm = W.copy()
            for o in range(K + 1):
                Wm[ridx, ridx + o] = np.nan
            wmin = np.nanmin(Wm, axis=1)
            wmax = np.nanmax(Wm, axis=1)
            row_min[g0 : g0 + 128] = np.minimum(s[:, 2], wmin)
            row_max[g0 : g0 + 128] = np.maximum(s[:, 3], wmax)
            m[g0 : g0 + 128] = W[ridx, ridx] * it  # exact diagonal row max
            Wv[g0 : g0 + 128] = W

    # global neg extremes of s = v*it - m_r
    neg_min = (row_min * it - m).min()
    neg_max = (row_max * it - m).max()
    a = 1.0 / (neg_max - neg_min + EPS)
    b_r = a * (cc - m - neg_min) + 1.0

    # pos/diag (pd) corrections from the raw windows
    rows = np.arange(B)
    r_in_blk = rows % 128
    pd_idx = r_in_blk[:, None] + np.arange(K + 1)[None, :]   # [B, 9] window cols
    v_pd = Wv[rows[:, None], pd_idx]                         # raw v at diag+pos
    s_pd = v_pd * it - cc                                    # s' = v*it - c
    E_pd = np.exp(s_pd)
    sum_pd_E = E_pd.sum(1)
    sum_pd_sE = (s_pd * E_pd).sum(1)

    A_all = it * sumU - cc * sumE            # sum s'E over all cols
    A_neg = A_all - sum_pd_sE
    B_neg = sumE - sum_pd_E

    Sw = a * A_neg + b_r * B_neg + sum_pd_E
    log_sw = np.log(Sw)

    # positive log-probs: pos o (o=1..K) of row r is window col r_in_blk+o
    v_pos = v_pd[:, 1:]                      # [B, K]
    pos_log = v_pos * it - cc - log_sw[:, None]

    pos_w = 1.0 - pos_vals.astype(np.float64)
    pos_w = (pos_w - pos_w.min()) / (pos_w.max() - pos_w.min() + EPS)
    loss = -np.mean(pos_log.reshape(-1) * pos_w)
    return np.float32(loss)


# revision 12
# speedup vs baseline: 23.8124x; 23.8124x over previous
"""Trainium2 Bass kernel for nn_ContrastiveLoss_82300163326281.

Strategy (8 NeuronCores, SPMD, no collectives):
  - Host rotates the embedding rows per core (core k gets roll(emb, -1024k))
    so every core runs the *same* program on its local rows 0..1023 while the
    full matrix column space is identical up to a permutation (row reductions
    are permutation invariant).
  - Device, per core:
      phase 0: normalize all B rows (ACT square-accum -> sqrt -> 1/x -> scale)
               and PE-transpose into a resident zT panel [2x128, B] (f32).
      main:    for each 128-row block x 2048-col chunk:
                 fp32r matmul -> PSUM (raw dots v)
                 ACT: E = exp(v*invtemp - c) with accum -> rowsum(E)
                 DVE: tensor_tensor_reduce -> rowsum(v*E)
                 DVE: rowwise min/max of v, skipping the 256-wide diagonal
                      window [128rb, 128rb+256) that contains the diagonal
                      and all K=8 positives  (c = 1/temp = the row max, since
                      the diagonal of a cosine-similarity matrix dominates)
                 DMA: ship the raw v window [128,256] to DRAM
  - Host finish (exact, f64): per-row masked min/max merge (device outside-
    window extremes + host window scan), global neg_min/neg_max, affine
    decomposition of the 'inverse_sim' weights  w = a*s' + b_r  so that
      sum_j w_j e^{s'_j} = a*sum(s'E) + b_r*sum(E) (+ pos/diag corrections),
    positive log-probs from the shipped windows, weighted mean.

Self-contained: hardcodes shapes; falls back to a pure-numpy replica of the
reference if the positive-index structure is not the expected banded pattern.
"""

import os
import sys

import numpy as np

sys.path.insert(0, "/opt/trn_rl_repo")

B = 8192
D = 256
K = 8
NCORES = 8
ROWS = B // NCORES          # 1024 rows per core
RB = ROWS // 128            # 8 row blocks per core
CHUNK = 2048
NCH = B // CHUNK            # 4 column chunks
WIN = 256                   # diagonal window width (>= 128 + K + 1)
EPS = 1e-8

_state = {}


# --------------------------------------------------------------------------
# device program
# --------------------------------------------------------------------------

def _build_program(invtemp: float, negc: float, use_ttr=True, use_f32r=True, do_main=True):
    from contextlib import ExitStack

    import concourse.bass as bass  # noqa: F401
    import concourse.mybir as mybir
    from concourse import bacc, tile

    f32 = mybir.dt.float32
    f32r = mybir.dt.float32r
    AF = mybir.ActivationFunctionType
    ALU = mybir.AluOpType
    AX = mybir.AxisListType

    nc = bacc.Bacc(
        "TRN2",
        target_bir_lowering=False,
        debug=False,
        num_devices=NCORES,
    )
    emb = nc.dram_tensor("emb", [B, D], f32, kind="ExternalInput").ap()
    stats = nc.dram_tensor("stats", [128, RB * 8], f32, kind="ExternalOutput").ap()
    wins = nc.dram_tensor("wins", [128, RB * WIN], f32, kind="ExternalOutput").ap()

    with tile.TileContext(nc) as tc, ExitStack() as ctx:
        const = ctx.enter_context(tc.tile_pool(name="const", bufs=1))
        ones = const.tile([128, 128], f32, tag="ones", name="ones")
        ident = const.tile([128, 128], f32, tag="ident", name="ident")
        ebias = const.tile([128, 1], f32, tag="ebias", name="ebias")
        nc.gpsimd.memset(ones[:], 1.0)
        nc.gpsimd.affine_select(
            ident[:],
            ones[:],
            pattern=[[1, 128]],
            compare_op=ALU.is_equal,
            fill=0.0,
            base=0,
            channel_multiplier=-1,
        )
        nc.gpsimd.memset(ebias[:], negc)

        ztp = ctx.enter_context(tc.tile_pool(name="ztp", bufs=1))
        # zt[:, 0:B] = dims 0..127 (chunk 0), zt[:, B:2B] = dims 128..255
        zt = ztp.tile([128, 2 * B], f32r if use_f32r else f32, tag="zt", name="zt")

        ep = ctx.enter_context(tc.tile_pool(name="ep", bufs=2))
        zp = ctx.enter_context(tc.tile_pool(name="zp", bufs=3))
        small = ctx.enter_context(tc.tile_pool(name="small", bufs=4))
        psum = ctx.enter_context(tc.tile_pool(name="psum", bufs=2, space="PSUM"))
        Epool = ctx.enter_context(tc.tile_pool(name="Epool", bufs=3))
        upool = ctx.enter_context(tc.tile_pool(name="upool", bufs=2))
        accp = ctx.enter_context(tc.tile_pool(name="accp", bufs=RB))
        outp = ctx.enter_context(tc.tile_pool(name="outp", bufs=1))

        stats_sb = outp.tile([128, RB * 8], f32, tag="stats_sb", name="stats_sb")
        nc.gpsimd.memset(stats_sb[:], 0.0)

        emb_r = emb.rearrange("(a p) d -> p a d", p=128)  # [128, 64, 256]

        def phase0(c):
            # normalize + transpose z row-tiles [16c, 16c+16)
            eg = ep.tile([128, 16 * D], f32, tag="eg", name=f"eg{c}")
            nc.sync.dma_start(
                out=eg[:].rearrange("p (a d) -> p a d", d=D),
                in_=emb_r[:, 16 * c : 16 * c + 16, :],
            )
            for j in range(8):  # 2 z-tiles per psum tile (4 banks)
                pt = psum.tile([128, CHUNK], f32, tag="pt", name=f"tp{c}_{j}")
                for h in range(2):
                    t = 16 * c + 2 * j + h
                    et = eg[:, (2 * j + h) * D : (2 * j + h + 1) * D]
                    sq = zp.tile([128, D], f32, tag="sq", name=f"sq{t}")
                    n2 = small.tile([128, 1], f32, tag="n2", name=f"n2_{t}")
                    nc.scalar.activation(sq[:], et, AF.Square, accum_out=n2[:])
                    nrm = small.tile([128, 1], f32, tag="nrm", name=f"nrm{t}")
                    nc.scalar.activation(nrm[:], n2[:], AF.Sqrt)
                    rn = small.tile([128, 1], f32, tag="rn", name=f"rn{t}")
                    nc.vector.reciprocal(rn[:], nrm[:])
                    zr = zp.tile([128, D], f32, tag="zr", name=f"zr{t}")
                    nc.scalar.activation(zr[:], et, AF.Copy, scale=rn[:])
                    # one transpose per PSUM bank (bank = one zero region)
                    nc.tensor.matmul(
                        pt[:, 1024 * h : 1024 * h + 128],
                        lhsT=zr[:, 0:128],
                        rhs=ident[:],
                        is_transpose=True,
                        start=True,
                        stop=True,
                    )
                    nc.tensor.matmul(
                        pt[:, 1024 * h + 512 : 1024 * h + 640],
                        lhsT=zr[:, 128:256],
                        rhs=ident[:],
                        is_transpose=True,
                        start=True,
                        stop=True,
                    )
                # evacuate PSUM -> zt panel (DMA cannot read PSUM); alternate
                # DVE / ACT so neither engine eats the whole copy cost
                t0 = 16 * c + 2 * j
                for h in range(2):
                    t = t0 + h
                    src0 = pt[:, 1024 * h : 1024 * h + 128]
                    src1 = pt[:, 1024 * h + 512 : 1024 * h + 640]
                    d0 = zt[:, 128 * t : 128 * t + 128]
                    d1 = zt[:, B + 128 * t : B + 128 * t + 128]
                    if t % 2 == 0:
                        nc.vector.tensor_copy(d0, src0)
                        nc.scalar.copy(d1, src1)
                    else:
                        nc.scalar.copy(d0, src0)
                        nc.vector.tensor_copy(d1, src1)

        def main_block(rb, c):
            pt = psum.tile([128, CHUNK], f32, tag="pt", name=f"pt{rb}_{c}")
            l0 = zt[:, 128 * rb : 128 * rb + 128]
            l1 = zt[:, B + 128 * rb : B + 128 * rb + 128]
            for b in range(CHUNK // 512):
                col = CHUNK * c + 512 * b
                nc.tensor.matmul(
                    pt[:, 512 * b : 512 * b + 512],
                    lhsT=l0,
                    rhs=zt[:, col : col + 512],
                    start=True,
                    stop=False,
                )
                nc.tensor.matmul(
                    pt[:, 512 * b : 512 * b + 512],
                    lhsT=l1,
                    rhs=zt[:, B + col : B + col + 512],
                    start=False,
                    stop=True,
                )

            se, su, mn, mx = _state["acc"][rb]
            E = Epool.tile([128, CHUNK], f32, tag="E", name=f"E{rb}_{c}")
            nc.scalar.activation(
                E[:],
                pt[:],
                AF.Exp,
                bias=ebias[:],
                scale=float(invtemp),
                accum_out=se[:, c : c + 1],
            )
            u = upool.tile([128, CHUNK], f32, tag="u", name=f"u{rb}_{c}")
            if use_ttr:
                nc.vector.scalar_tensor_tensor(
                    out=u[:],
                    in0=pt[:],
                    scalar=1.0,
                    in1=E[:],
                    op0=ALU.bypass,
                    op1=ALU.mult,
                    accum_out=su[:, c : c + 1],
                )
            else:
                nc.vector.tensor_tensor(u[:], pt[:], E[:], op=ALU.mult)
                nc.vector.reduce_sum(su[:, c : c + 1], u[:], axis=AX.X)

            # min/max of raw v, excluding the diagonal window on chunk 0
            if c == 0:
                o = 128 * rb
                pieces = []
                if rb > 0:
                    pieces.append((0, o))
                pieces.append((o + WIN, CHUNK - (o + WIN)))
                wstage = Epool.tile(
                    [128, WIN], f32, tag="wstage", name=f"wstage{rb}", bufs=2
                )
                nc.scalar.copy(wstage[:], pt[:, o : o + WIN])
                nc.sync.dma_start(
                    out=wins[:, WIN * rb : WIN * rb + WIN],
                    in_=wstage[:],
                )
            else:
                pieces = [(0, CHUNK)]
            pidx = _state["pidx"][rb]
            for (a, w) in pieces:
                nc.vector.tensor_reduce(
                    mn[:, pidx : pidx + 1], pt[:, a : a + w], axis=AX.X, op=ALU.min
                )
                nc.vector.tensor_reduce(
                    mx[:, pidx : pidx + 1], pt[:, a : a + w], axis=AX.X, op=ALU.max
                )
                pidx += 1
            _state["pidx"][rb] = pidx

        def finish_block(rb):
            se, su, mn, mx = _state["acc"][rb]
            npieces = _state["pidx"][rb]
            nc.vector.tensor_reduce(
                stats_sb[:, 8 * rb + 0 : 8 * rb + 1], se[:], axis=AX.X, op=ALU.add
            )
            nc.vector.tensor_reduce(
                stats_sb[:, 8 * rb + 1 : 8 * rb + 1 + 1], su[:], axis=AX.X, op=ALU.add
            )
            nc.vector.tensor_reduce(
                stats_sb[:, 8 * rb + 2 : 8 * rb + 3],
                mn[:, 0:npieces],
                axis=AX.X,
                op=ALU.min,
            )
            nc.vector.tensor_reduce(
                stats_sb[:, 8 * rb + 3 : 8 * rb + 4],
                mx[:, 0:npieces],
                axis=AX.X,
                op=ALU.max,
            )

        # per-rowblock accumulators
        _state["acc"] = {}
        _state["pidx"] = {}
        for rb in range(RB):
            se = accp.tile([128, NCH], f32, tag="se", name=f"se{rb}")
            su = accp.tile([128, NCH], f32, tag="su", name=f"su{rb}")
            mn = accp.tile([128, 5], f32, tag="mn", name=f"mn{rb}")
            mx = accp.tile([128, 5], f32, tag="mx", name=f"mx{rb}")
            _state["acc"][rb] = (se, su, mn, mx)
            _state["pidx"][rb] = 0

        phase0(0)
        for c in range(NCH):
            if c + 1 < NCH:
                phase0(c + 1)
            if do_main:
                for rb in range(RB):
                    main_block(rb, c)
        if do_main:
            for rb in range(RB):
                finish_block(rb)

        nc.sync.dma_start(out=stats, in_=stats_sb[:])

        _state.pop("acc", None)
        _state.pop("pidx", None)

    nc.compile()
    return nc


# --------------------------------------------------------------------------
# runners
# --------------------------------------------------------------------------

def _get_program(invtemp: float, negc: float):
    key = ("prog", float(invtemp), float(negc))
    if key not in _state:
        _state[key] = _build_program(invtemp, negc)
    return _state[key]


def _run_device_stock(nc, in_maps):
    from concourse.bass_utils import run_bass_kernel_spmd

    res = run_bass_kernel_spmd(nc, in_maps, list(range(NCORES)))
    _state["last_results"] = res
    return res.results


def _make_cached_runner(nc, return_parts=False):
    """Vendored multi-core tail of bass2jax.run_bass_via_pjrt, but keeping the
    jitted callable so repeated invocations (for timing) do not recompile."""
    import jax
    import concourse.mybir as mybir
    from jax.sharding import Mesh, PartitionSpec
    from concourse.bass2jax import (
        _bass_exec_p,
        install_neuronx_cc_hook,
        partition_id_tensor,
    )

    try:
        from jax.experimental.shard_map import shard_map
    except Exception:  # newer jax
        from jax import shard_map  # type: ignore

    install_neuronx_cc_hook()

    partition_name = nc.partition_id_tensor.name if nc.partition_id_tensor else None
    in_names, out_names, out_avals, zero_outs = [], [], [], []
    for alloc in nc.m.functions[0].allocations:
        if not isinstance(alloc, mybir.MemoryLocationSet):
            continue
        name = alloc.memorylocations[0].name
        if alloc.kind == "ExternalInput":
            if name != partition_name:
                in_names.append(name)
        elif alloc.kind == "ExternalOutput":
            out_names.append(name)
            shape = tuple(alloc.tensor_shape)
            dtype = mybir.dt.np(alloc.dtype)
            out_avals.append(jax.core.ShapedArray(shape, dtype))
            zero_outs.append(np.zeros(shape, dtype))
    n_params = len(in_names)
    all_names = in_names + out_names
    if partition_name is not None:
        all_names = all_names + [partition_name]
    donate = tuple(range(n_params, n_params + len(out_names)))

    def _body(*args):
        operands = list(args)
        if partition_name is not None:
            operands.append(partition_id_tensor())
        outs = _bass_exec_p.bind(
            *operands,
            out_avals=tuple(out_avals),
            in_names=tuple(all_names),
            out_names=tuple(out_names),
            lowering_input_output_aliases=(),
            sim_require_finite=True,
            sim_require_nnan=True,
            nc=nc,
        )
        return tuple(outs)

    devices = jax.devices()[:NCORES]
    mesh = Mesh(np.asarray(devices), ("core",))
    n_out = len(out_names)
    sharded = jax.jit(
        shard_map(
            _body,
            mesh=mesh,
            in_specs=(PartitionSpec("core"),) * (n_params + n_out),
            out_specs=(PartitionSpec("core"),) * n_out,
            check_rep=False,
        ),
        donate_argnums=donate,
        keep_unused=True,
    )

    def run(in_maps):
        concat_in = [
            np.concatenate([np.asarray(m[nm]) for m in in_maps], axis=0)
            for nm in in_names
        ]
        concat_zeros = [
            np.zeros((NCORES * z.shape[0], *z.shape[1:]), z.dtype) for z in zero_outs
        ]
        out_arrs = sharded(*concat_in, *concat_zeros)
        return [
            {
                nm: np.asarray(out_arrs[i]).reshape(NCORES, *out_avals[i].shape)[c]
                for i, nm in enumerate(out_names)
            }
            for c in range(NCORES)
        ]

    if return_parts:
        return run, sharded, in_names, out_avals, zero_outs
    return run


def _run_device(nc, in_maps):
    if os.environ.get("KERNEL_FAST_RUNNER"):
        key = ("runner", id(nc))
        if key not in _state:
            _state[key] = _make_cached_runner(nc)
        return _state[key](in_maps)
    return _run_device_stock(nc, in_maps)


# --------------------------------------------------------------------------
# host finish
# --------------------------------------------------------------------------

def _numpy_reference(emb, pos_vals, temperature, pos_row, pos_col):
    """Exact fallback replica of the reference (used only if the positive
    index pattern is not the expected banded structure)."""
    n = emb.shape[0]
    norm = np.sqrt((emb.astype(np.float32) ** 2).sum(1, keepdims=True))
    z = emb / np.maximum(norm, np.float32(1e-12))
    temp = np.float32(np.log1p(np.exp(np.float64(temperature))))
    sim = (z @ z.T) / temp
    sim = sim - sim.max(axis=1, keepdims=True)
    posd = np.zeros((n, n), bool)
    posd[pos_row, pos_col] = True
    negm = ~posd & ~np.eye(n, dtype=bool)
    pos_w = 1.0 - pos_vals
    pos_w = (pos_w - pos_w.min()) / (pos_w.max() - pos_w.min() + np.float32(EPS))
    neg_min = sim[negm].min()
    neg_max = sim[negm].max()
    neg_w = (sim - neg_min) / (neg_max - neg_min + np.float32(EPS)) + 1.0
    logw = np.where(negm, np.log(neg_w), 0.0).astype(np.float32)
    a = (sim + logw).astype(np.float64)
    lse = np.log(np.exp(a).sum(1))
    pl = sim[pos_row, pos_col].astype(np.float64) - lse[pos_row]
    return np.float32(-np.mean(pl * pos_w.astype(np.float64)))


def kernel(**inputs):
    emb = np.ascontiguousarray(np.asarray(inputs["embeddings"], dtype=np.float32))
    pos_vals = np.asarray(inputs["pos_vals"], dtype=np.float32)
    temperature = np.asarray(inputs["temperature"], dtype=np.float32)
    pos_row = np.asarray(inputs["pos_row"]).astype(np.int64)
    pos_col = np.asarray(inputs["pos_col"]).astype(np.int64)

    rr = np.repeat(np.arange(B, dtype=np.int64), K)
    oo = np.tile(np.arange(1, K + 1, dtype=np.int64), B)
    structured = (
        emb.shape == (B, D)
        and pos_row.shape == (B * K,)
        and np.array_equal(pos_row, rr)
        and np.array_equal(pos_col, (rr + oo) % B)
    )
    if not structured:
        return _numpy_reference(emb, pos_vals, temperature, pos_row, pos_col)

    temp = float(np.log1p(np.exp(np.float64(temperature))))
    invtemp = 1.0 / np.float32(temp)  # f32 to match device immediates
    invtemp = float(np.float32(invtemp))
    c = invtemp  # row max == diagonal == 1/temp
    negc = float(np.float32(-c))

    nc = _get_program(invtemp, negc)
    in_maps = [
        {"emb": np.roll(emb, -ROWS * k, axis=0)} for k in range(NCORES)
    ]
    results = _run_device(nc, in_maps)

    # ---- host finish (f64) ----
    it = np.float64(invtemp)
    cc = np.float64(c)

    sumE = np.empty(B)
    sumU = np.empty(B)
    row_min = np.empty(B)
    row_max = np.empty(B)
    m = np.empty(B)
    Wv = np.empty((B, WIN))

    ridx = np.arange(128)
    for k in range(NCORES):
        stats = results[k]["stats"].astype(np.float64)  # [128, RB*8]
        wins = results[k]["wins"].astype(np.float64)    # [128, RB*WIN]
        for rb in range(RB):
            g0 = ROWS * k + 128 * rb
            s = stats[:, 8 * rb : 8 * rb + 8]
            W = wins[:, WIN * rb : WIN * rb + WIN]  # [128, 256] raw v
            sumE[g0 : g0 + 128] = s[:, 0]
            sumU[g0 : g0 + 128] = s[:, 1]
            # masked min/max inside window: exclude relative cols r..r+K
            Wm = W.copy()
            for o in range(K + 1):
                Wm[ridx, ridx + o] = np.nan
            wmin = np.nanmin(Wm, axis=1)
            wmax = np.nanmax(Wm, axis=1)
            row_min[g0 : g0 + 128] = np.minimum(s[:, 2], wmin)
            row_max[g0 : g0 + 128] = np.maximum(s[:, 3], wmax)
            m[g0 : g0 + 128] = W[ridx, ridx] * it  # exact diagonal row max
            Wv[g0 : g0 + 128] = W

    # global neg extremes of s = v*it - m_r
    neg_min = (row_min * it - m).min()
    neg_max = (row_max * it - m).max()
    a = 1.0 / (neg_max - neg_min + EPS)
    b_r = a * (cc - m - neg_min) + 1.0

    # pos/diag (pd) corrections from the raw windows
    rows = np.arange(B)
    r_in_blk = rows % 128
    pd_idx = r_in_blk[:, None] + np.arange(K + 1)[None, :]   # [B, 9] window cols
    v_pd = Wv[rows[:, None], pd_idx]                         # raw v at diag+pos
    s_pd = v_pd * it - cc                                    # s' = v*it - c
    E_pd = np.exp(s_pd)
    sum_pd_E = E_pd.sum(1)
    sum_pd_sE = (s_pd * E_pd).sum(1)

    A_all = it * sumU - cc * sumE            # sum s'E over all cols
    A_neg = A_all - sum_pd_sE
    B_neg = sumE - sum_pd_E

    Sw = a * A_neg + b_r * B_neg + sum_pd_E
    log_sw = np.log(Sw)

    # positive log-probs: pos o (o=1..K) of row r is window col r_in_blk+o
    v_pos = v_pd[:, 1:]                      # [B, K]
    pos_log = v_pos * it - cc - log_sw[:, None]

    pos_w = 1.0 - pos_vals.astype(np.float64)
    pos_w = (pos_w - pos_w.min()) / (pos_w.max() - pos_w.min() + EPS)
    loss = -np.mean(pos_log.reshape(-1) * pos_w)
    return np.float32(loss)
